# revision 9
# baseline (speedup 1.0000x reference)
"""BERTgrid generator kernel for Trainium2 (8 NeuronCores, batch-parallel).

Per core (one document):
  emb [512, 768] f32, coors [512, 4] i32, mask [512, 1] i32
  -> out [768, 128*96] f32   (channel-major grid)

Device algorithm (no host compute on input values):
  1. valid/new_word/seg via triangular-matmul cumsums.
  2. Word mean table (shifted by one word) via one-hot matmul + reciprocal.
  3. Per-pixel last-covering-word index via two exponent-weighted matmuls:
     S1 = sum_words 128^(seg//32) over covering boxes  -> max chunk via f32
     exponent field; M_k = sum_words 4^(seg%32) per chunk -> max offset.
     All index math is exact (integer ops on the exponent bits).
  4. Paint: out[d, p] = table[widx[p], d] as table^T @ onehot(widx) in fp16
     (one-hot has a single 1 per column, so fp16 only rounds table values).
"""

import sys

import numpy as np

try:
    import concourse.bass as bass
except ImportError:  # grading env fallback
    sys.path.insert(0, "/opt/trn_rl_repo")
    import concourse.bass as bass

from concourse import bacc
import concourse.tile as tile
from concourse import mybir
from concourse.bass_utils import run_bass_kernel_spmd
from contextlib import ExitStack

P = 128
S, D = 512, 768
R, C, STRIDE = 128, 96, 8
T = S // P            # token tiles
WT = S // P           # word tiles / k-chunks
NPIX = R * C          # 12288
PG = 1536             # pixels per paint group
NG = NPIX // PG
DT = D // P

F32 = mybir.dt.float32
F16 = mybir.dt.float16
BF16 = mybir.dt.bfloat16
I32 = mybir.dt.int32
OP = mybir.AluOpType

_last_results = None


def _build():
    nc = bacc.Bacc(None, target_bir_lowering=False)
    emb_ext = nc.declare_dram_parameter("emb", [S, D], F32, isOutput=False)
    coors_ext = nc.declare_dram_parameter("coors", [S, 4], I32, isOutput=False)
    mask_ext = nc.declare_dram_parameter("mask", [S, 1], I32, isOutput=False)
    out_ext = nc.declare_dram_parameter("out", [D, NPIX], F32, isOutput=True)
    widx_dram = nc.dram_tensor("widx_scratch", [P, C], F16)

    with tile.TileContext(nc) as tc, ExitStack() as ctx:
        sing = ctx.enter_context(tc.tile_pool(name="sing", bufs=1))

        # ---- constants ----
        def iota_tile(name, shape, pattern, base, cm, out_dt=F32):
            it = sing.tile(shape, I32, tag=name + "_i")
            nc.gpsimd.iota(it[:], pattern, base=base, channel_multiplier=cm)
            if out_dt == I32:
                return it
            ft = sing.tile(shape, out_dt, tag=name)
            nc.vector.tensor_copy(ft[:], it[:])
            return ft

        iota_r = iota_tile("iota_r", [P, R], [[1, R]], 0, 0)          # 0..127 along free
        iota_c = iota_tile("iota_c", [P, C], [[1, C]], 0, 0)          # 0..95
        iota16 = iota_tile("iota16", [P, 16], [[1, 16]], 0, 0)        # 0..15
        iotaW = [iota_tile(f"iotaW{wt}", [P, P], [[1, P]], wt * P - 1, 0)
                 for wt in range(WT)]                                  # word-1 values
        iotawp = [iota_tile(f"iotawp{kc}", [P, 1], [[0, 1]], kc * P, 1)
                  for kc in range(WT)]                                 # per-partition word id

        tri_i = sing.tile([P, P], I32, tag="tri_i")
        nc.gpsimd.iota(tri_i[:], [[1, P]], base=0, channel_multiplier=-1)  # i - j
        tri_f = sing.tile([P, P], F32, tag="tri_f")
        nc.vector.tensor_copy(tri_f[:], tri_i[:])
        tri = sing.tile([P, P], BF16, tag="tri")                       # [j, i] = (j <= i)
        nc.vector.tensor_scalar(out=tri[:], in0=tri_f[:], scalar1=0.0,
                                scalar2=None, op0=OP.is_ge)
        ones_bf = sing.tile([P, P], BF16, tag="ones_bf")
        nc.vector.memset(ones_bf[:], 1.0)

        # ---- token-tile loads + simple per-token quantities ----
        mask_t, coors_t, coorsm1_t = [], [], []
        invm_bf, same_f, wcf = [], [], []
        embext = []
        for t in range(T):
            mt_ = sing.tile([P, 1], I32, tag=f"mask{t}")
            nc.sync.dma_start(out=mt_[:], in_=mask_ext[t * P:(t + 1) * P, :])
            mask_t.append(mt_)
            ct = sing.tile([P, 4], I32, tag=f"coors{t}")
            nc.sync.dma_start(out=ct[:], in_=coors_ext[t * P:(t + 1) * P, :])
            coors_t.append(ct)
            cm1 = sing.tile([P, 4], I32, tag=f"coorsm1{t}")
            if t == 0:
                nc.vector.memset(cm1[:], -1)
                nc.sync.dma_start(out=cm1[1:P, :], in_=coors_ext[0:P - 1, :])
            else:
                nc.sync.dma_start(out=cm1[:], in_=coors_ext[t * P - 1:(t + 1) * P - 1, :])
            coorsm1_t.append(cm1)
            et = sing.tile([P, D + 1], F16, tag=f"emb{t}")
            nc.vector.memset(et[:, D:D + 1], 1.0)
            nc.gpsimd.dma_start(out=et[:, 0:D], in_=emb_ext[t * P:(t + 1) * P, :])
            embext.append(et)

        for t in range(T):
            mf = sing.tile([P, 1], F32, tag=f"maskf{t}")
            nc.vector.tensor_copy(mf[:], mask_t[t][:])
            ib = sing.tile([P, 1], BF16, tag=f"invm{t}")
            nc.vector.tensor_scalar(out=ib[:], in0=mf[:], scalar1=1.0,
                                    scalar2=-1.0, op0=OP.subtract, op1=OP.mult)
            invm_bf.append(ib)
            cf = sing.tile([P, 4], F32, tag=f"coorsf{t}")
            nc.vector.tensor_copy(cf[:], coors_t[t][:])
            cm1f = sing.tile([P, 4], F32, tag=f"coorsm1f{t}")
            nc.vector.tensor_copy(cm1f[:], coorsm1_t[t][:])
            eq4 = sing.tile([P, 4], F32, tag=f"eq4{t}")
            nc.vector.tensor_tensor(eq4[:], cf[:], cm1f[:], OP.is_equal)
            sf = sing.tile([P, 1], F32, tag=f"same{t}")
            nc.vector.tensor_reduce(sf[:], eq4[:], mybir.AxisListType.X, OP.min)
            same_f.append(sf)
            wi_ = sing.tile([P, 4], I32, tag=f"wci{t}")
            nc.vector.tensor_scalar(out=wi_[:], in0=coors_t[t][:], scalar1=3,
                                    scalar2=None, op0=OP.arith_shift_right)
            wf = sing.tile([P, 4], F32, tag=f"wc{t}")
            nc.vector.tensor_copy(wf[:], wi_[:])
            wcf.append(wf)

        # ---- valid = (cumsum(1-mask) == 0) ----
        valid_f, nw_f, nw_bf = [], [], []
        with tc.tile_pool(name="psA", bufs=1, space="PSUM") as psA:
            vps = []
            for mt in range(T):
                vp = psA.tile([P, 1], F32, tag=f"v{mt}")
                for kc in range(mt + 1):
                    nc.tensor.matmul(out=vp[:],
                                     lhsT=(tri[:] if kc == mt else ones_bf[:]),
                                     rhs=invm_bf[kc][:],
                                     start=(kc == 0), stop=(kc == mt))
                vps.append(vp)
            for t in range(T):
                vf = sing.tile([P, 1], F32, tag=f"valid{t}")
                nc.vector.tensor_scalar(out=vf[:], in0=vps[t][:], scalar1=0.5,
                                        scalar2=None, op0=OP.is_lt)
                valid_f.append(vf)
                nwf = sing.tile([P, 1], F32, tag=f"nw{t}")
                nc.vector.scalar_tensor_tensor(out=nwf[:], in0=same_f[t][:],
                                               scalar=0.5, in1=vf[:],
                                               op0=OP.is_lt, op1=OP.mult)
                nw_f.append(nwf)
                nwb = sing.tile([P, 1], BF16, tag=f"nwb{t}")
                nc.vector.tensor_copy(nwb[:], nwf[:])
                nw_bf.append(nwb)

        # ---- seg = cumsum(new_word) - 1 ----
        seg_f, seg_i = [], []
        with tc.tile_pool(name="psB", bufs=1, space="PSUM") as psB:
            sps = []
            for mt in range(T):
                sp = psB.tile([P, 1], F32, tag=f"s{mt}")
                for kc in range(mt + 1):
                    nc.tensor.matmul(out=sp[:],
                                     lhsT=(tri[:] if kc == mt else ones_bf[:]),
                                     rhs=nw_bf[kc][:],
                                     start=(kc == 0), stop=(kc == mt))
                sps.append(sp)
            for t in range(T):
                sf = sing.tile([P, 1], F32, tag=f"seg{t}")
                nc.vector.tensor_scalar(out=sf[:], in0=sps[t][:], scalar1=1.0,
                                        scalar2=None, op0=OP.subtract)
                seg_f.append(sf)
                si = sing.tile([P, 1], I32, tag=f"segi{t}")
                nc.vector.tensor_copy(si[:], sf[:])
                seg_i.append(si)

        # ---- per-token scan weights + coverage masks ----
        rowcov, rhs1, rhs2 = [], [], []
        for t in range(T):
            chunk_i = sing.tile([P, 1], I32, tag=f"chunk{t}")
            nc.vector.tensor_scalar(out=chunk_i[:], in0=seg_i[t][:], scalar1=5,
                                    scalar2=None, op0=OP.arith_shift_right)
            chunk_fl = sing.tile([P, 1], F32, tag=f"chunkf{t}")
            nc.vector.tensor_copy(chunk_fl[:], chunk_i[:])
            # w1 = 2^(7*chunk) built from exponent bits
            w1b = sing.tile([P, 1], I32, tag=f"w1b{t}")
            nc.vector.tensor_scalar(out=w1b[:], in0=chunk_i[:], scalar1=7,
                                    scalar2=127, op0=OP.mult, op1=OP.add)
            nc.vector.tensor_scalar(out=w1b[:], in0=w1b[:], scalar1=23,
                                    scalar2=None, op0=OP.logical_shift_left)
            cw1 = sing.tile([P, 1], F32, tag=f"cw1{t}")
            nc.vector.tensor_tensor(cw1[:], w1b[:].bitcast(F32), nw_f[t][:], OP.mult)
            # w2 = 2^(2*(seg&31))
            w2b = sing.tile([P, 1], I32, tag=f"w2b{t}")
            nc.vector.tensor_scalar(out=w2b[:], in0=seg_i[t][:], scalar1=31,
                                    scalar2=None, op0=OP.bitwise_and)
            nc.vector.tensor_scalar(out=w2b[:], in0=w2b[:], scalar1=1,
                                    scalar2=None, op0=OP.logical_shift_left)
            nc.vector.tensor_scalar(out=w2b[:], in0=w2b[:], scalar1=127,
                                    scalar2=None, op0=OP.add)
            nc.vector.tensor_scalar(out=w2b[:], in0=w2b[:], scalar1=23,
                                    scalar2=None, op0=OP.logical_shift_left)
            cw2 = sing.tile([P, 1], F32, tag=f"cw2{t}")
            nc.vector.tensor_tensor(cw2[:], w2b[:].bitcast(F32), nw_f[t][:], OP.mult)

            y0, y1 = wcf[t][:, 1:2], wcf[t][:, 3:4]
            x0, x1 = wcf[t][:, 0:1], wcf[t][:, 2:3]
            tge = sing.tile([P, R], F32, tag=f"tge{t}")
            nc.vector.tensor_scalar(out=tge[:], in0=iota_r[:], scalar1=y0,
                                    scalar2=None, op0=OP.is_ge)
            rc = sing.tile([P, R], BF16, tag=f"rowcov{t}")
            nc.vector.scalar_tensor_tensor(out=rc[:], in0=iota_r[:], scalar=y1,
                                           in1=tge[:], op0=OP.is_lt, op1=OP.mult)
            rowcov.append(rc)

            cge = sing.tile([P, C], F32, tag=f"cge{t}")
            nc.vector.tensor_scalar(out=cge[:], in0=iota_c[:], scalar1=x0,
                                    scalar2=None, op0=OP.is_ge)
            ccv = sing.tile([P, C], F32, tag=f"colcov{t}")
            nc.vector.scalar_tensor_tensor(out=ccv[:], in0=iota_c[:], scalar=x1,
                                           in1=cge[:], op0=OP.is_lt, op1=OP.mult)

            r1 = sing.tile([P, C], BF16, tag=f"rhs1{t}")
            nc.vector.tensor_scalar(out=r1[:], in0=ccv[:], scalar1=cw1[:, 0:1],
                                    scalar2=None, op0=OP.mult)
            rhs1.append(r1)

            tmp16 = sing.tile([P, 16], F32, tag=f"tmp16{t}")
            nc.vector.tensor_scalar(out=tmp16[:], in0=iota16[:],
                                    scalar1=chunk_fl[:, 0:1], scalar2=cw2[:, 0:1],
                                    op0=OP.is_equal, op1=OP.mult)
            r2 = sing.tile([P, 16 * C], BF16, tag=f"rhs2{t}")
            nc.vector.tensor_tensor(
                r2[:].rearrange("p (a b) -> p a b", a=16),
                tmp16[:].unsqueeze(2).broadcast_to([P, 16, C]),
                ccv[:].unsqueeze(1).broadcast_to([P, 16, C]),
                OP.mult)
            rhs2.append(r2)

        # ---- index map via stage matmuls ----
        widx16 = sing.tile([P, C], F16, tag="widx16")
        with tc.tile_pool(name="psC", bufs=1, space="PSUM") as psC:
            ps1 = psC.tile([P, C], F32, tag="ps1")
            for kc in range(T):
                nc.tensor.matmul(out=ps1[:], lhsT=rowcov[kc][:], rhs=rhs1[kc][:],
                                 start=(kc == 0), stop=(kc == T - 1))
            ps2 = psC.tile([P, 16 * C], F32, tag="ps2")
            for n3 in range(3):
                sl = slice(n3 * 512, (n3 + 1) * 512)
                for kc in range(T):
                    nc.tensor.matmul(out=ps2[:, sl], lhsT=rowcov[kc][:],
                                     rhs=rhs2[kc][:, sl],
                                     start=(kc == 0), stop=(kc == T - 1))

            s1m = sing.tile([P, C], F32, tag="s1m")
            nc.vector.tensor_scalar(out=s1m[:], in0=ps1[:], scalar1=1.0,
                                    scalar2=None, op0=OP.max)
            e1 = sing.tile([P, C], I32, tag="e1")
            nc.vector.tensor_scalar(out=e1[:], in0=s1m[:].bitcast(I32), scalar1=23,
                                    scalar2=None, op0=OP.logical_shift_right)
            nc.vector.tensor_scalar(out=e1[:], in0=e1[:], scalar1=127,
                                    scalar2=None, op0=OP.subtract)
            cst_i = sing.tile([P, C], I32, tag="cst_i")
            nc.vector.tensor_scalar(out=cst_i[:], in0=e1[:], scalar1=9363,
                                    scalar2=None, op0=OP.mult)
            nc.vector.tensor_scalar(out=cst_i[:], in0=cst_i[:], scalar1=16,
                                    scalar2=None, op0=OP.arith_shift_right)
            cst_f = sing.tile([P, C], F32, tag="cst_f")
            nc.vector.tensor_copy(cst_f[:], cst_i[:])

            msel = sing.tile([P, C], F32, tag="msel")
            nc.vector.memset(msel[:], 0.0)
            mk = sing.tile([P, C], F32, tag="mk")
            tk = sing.tile([P, C], F32, tag="tk")
            for k in range(16):
                nc.vector.tensor_scalar(out=mk[:], in0=cst_f[:], scalar1=float(k),
                                        scalar2=None, op0=OP.is_equal)
                nc.vector.tensor_tensor(tk[:], mk[:], ps2[:, k * C:(k + 1) * C],
                                        OP.mult)
                nc.vector.tensor_tensor(msel[:], msel[:], tk[:], OP.add)

            mm = sing.tile([P, C], F32, tag="mm")
            nc.vector.tensor_scalar(out=mm[:], in0=msel[:], scalar1=1.0,
                                    scalar2=None, op0=OP.max)
            e2 = sing.tile([P, C], I32, tag="e2")
            nc.vector.tensor_scalar(out=e2[:], in0=mm[:].bitcast(I32), scalar1=23,
                                    scalar2=None, op0=OP.logical_shift_right)
            nc.vector.tensor_scalar(out=e2[:], in0=e2[:], scalar1=127,
                                    scalar2=None, op0=OP.subtract)
            lo = sing.tile([P, C], I32, tag="lo")
            nc.vector.tensor_scalar(out=lo[:], in0=e2[:], scalar1=1,
                                    scalar2=None, op0=OP.arith_shift_right)
            wi = sing.tile([P, C], I32, tag="wi")
            nc.vector.tensor_scalar(out=wi[:], in0=cst_i[:], scalar1=5,
                                    scalar2=None, op0=OP.logical_shift_left)
            nc.vector.tensor_tensor(wi[:], wi[:], lo[:], OP.add)
            nc.vector.tensor_copy(widx16[:], wi[:])

        # round-trip through DRAM to flatten + broadcast across partitions
        nc.sync.dma_start(out=widx_dram[:], in_=widx16[:])
        widx_bc = sing.tile([P, NPIX], F16, tag="widx_bc")
        nc.gpsimd.dma_start(
            out=widx_bc[:],
            in_=widx_dram[:].rearrange("p c -> (p c)").partition_broadcast(P))

        # ---- word mean table (shifted by one word) ----
        # O'[i, w] = valid[i] * (seg[i] == w - 1); table[w] = sum/cnt, row 0 = 0
        table16 = []
        Opr = [[None] * WT for _ in range(T)]
        for t in range(T):
            for wt in range(WT):
                o = sing.tile([P, P], F16, tag=f"op{t}_{wt}")
                nc.vector.tensor_scalar(out=o[:], in0=iotaW[wt][:],
                                        scalar1=seg_f[t][:, 0:1],
                                        scalar2=valid_f[t][:, 0:1],
                                        op0=OP.is_equal, op1=OP.mult)
                Opr[t][wt] = o
        with tc.tile_pool(name="psD", bufs=2, space="PSUM") as psD:
            for wt in range(WT):
                ptab = psD.tile([P, 1024], F32, tag="ptab")
                for kc in range(T):
                    nc.tensor.matmul(out=ptab[:, 0:512], lhsT=Opr[kc][wt][:],
                                     rhs=embext[kc][:, 0:512],
                                     start=(kc == 0), stop=(kc == T - 1))
                    nc.tensor.matmul(out=ptab[:, 512:D + 1], lhsT=Opr[kc][wt][:],
                                     rhs=embext[kc][:, 512:D + 1],
                                     start=(kc == 0), stop=(kc == T - 1))
                rec = sing.tile([P, 1], F32, tag="rec")
                nc.vector.tensor_scalar(out=rec[:], in0=ptab[:, D:D + 1],
                                        scalar1=1.0, scalar2=None, op0=OP.max)
                recr = sing.tile([P, 1], F32, tag="recr")
                nc.vector.reciprocal(recr[:], rec[:])
                tb = sing.tile([P, D], F16, tag=f"table{wt}")
                nc.vector.tensor_scalar(out=tb[:], in0=ptab[:, 0:D],
                                        scalar1=recr[:, 0:1], scalar2=None,
                                        op0=OP.mult)
                table16.append(tb)

        # ---- paint: out[d, p] = table[widx[p], d] ----
        with tc.tile_pool(name="oh", bufs=2) as ohp, \
             tc.tile_pool(name="stage", bufs=3) as stp, \
             tc.tile_pool(name="pp", bufs=2, space="PSUM") as ppp:
            for g in range(NG):
                gs = slice(g * PG, (g + 1) * PG)
                ohs = []
                for kc in range(WT):
                    oh = ohp.tile([P, PG], F16, tag=f"oh{kc}")
                    nc.vector.tensor_scalar(out=oh[:], in0=widx_bc[:, gs],
                                            scalar1=iotawp[kc][:, 0:1],
                                            scalar2=None, op0=OP.is_equal)
                    ohs.append(oh)
                for dt in range(DT):
                    stage = stp.tile([P, PG], F32, tag="stage")
                    pps = [ppp.tile([P, 512], F32, tag=f"pp{s}", name=f"pp{s}")
                           for s in range(3)]
                    for kc in range(WT):
                        dsl = slice(dt * P, (dt + 1) * P)
                        for s3 in range(3):
                            nc.tensor.matmul(
                                out=pps[s3][:], lhsT=table16[kc][:, dsl],
                                rhs=ohs[kc][:, s3 * 512:(s3 + 1) * 512],
                                start=(kc == 0), stop=(kc == WT - 1))
                    for s3 in range(3):
                        nc.any.tensor_copy(out=stage[:, s3 * 512:(s3 + 1) * 512],
                                           in_=pps[s3][:])
                    nc.sync.dma_start(out=out_ext[dt * P:(dt + 1) * P, gs],
                                      in_=stage[:])
    nc.compile()
    return nc


_nc_cache = None


def kernel(bert_embeddings, coors, mask, image_h=1024, image_w=768, stride=8):
    global _last_results, _nc_cache
    emb = np.ascontiguousarray(np.asarray(bert_embeddings, dtype=np.float32))
    co = np.ascontiguousarray(np.asarray(coors, dtype=np.int32))
    mk = np.ascontiguousarray(np.asarray(mask, dtype=np.int32))
    ih, iw, st = int(image_h), int(image_w), int(stride)
    B = emb.shape[0]
    assert (ih // st, iw // st) == (R, C) and st == STRIDE
    assert emb.shape == (B, S, D) and B == 8

    if _nc_cache is None:
        _nc_cache = _build()
    nc = _nc_cache

    in_maps = [{"emb": emb[b], "coors": co[b], "mask": mk[b].reshape(S, 1)}
               for b in range(B)]
    res = run_bass_kernel_spmd(nc, in_maps, core_ids=list(range(B)))
    _last_results = res
    out = np.stack([np.asarray(res.results[b]["out"]).reshape(D, R, C)
                    for b in range(B)])
    return out.astype(np.float32)
